# revision 13
# baseline (speedup 1.0000x reference)
"""Trainium2 Bass kernel for causal self-attention with RoPE.

Shapes: x (2, 2048, 2048), 16 heads x 128 head_dim.
Sharding: 8 cores = 2 batch x 4 head-groups (4 heads per core).
Each core computes q/k/v projections for its heads, RoPE, causal-masked
softmax attention, and a partial output projection (its head columns of
wo); the host sums the 4 partials per batch element.

Layout strategy (per core):
  - q,k built in transposed layout (head_dim on partitions, t free) so
    RoPE and the score matmuls need no on-device transposes.  The host
    permutes wq/wk columns so RoPE's even/odd pairs become the two
    partition halves, and pre-scales wq by 1/sqrt(head_dim).
  - scores computed as s^T (keys x q) per 256-query group; softmax skips
    the max-subtraction (scores are O(1) by construction); row sums via
    an ones-vector matmul; normalization folded into the PSUM eviction.
  - v computed directly in (t x e) layout by using x^T as the stationary
    operand, so the p@v matmul needs no transposes anywhere.
  - fully-masked key blocks are skipped (host inspects the mask), which
    halves the attention work for the causal mask; deduplicated mask
    tiles are added only where a block is partially masked.
  - matmuls run in float32r (tf32) which streams at full rate for moving
    dims >= 256.
  - weights/mask stream on the scalar-engine DMA queue, x^T/tables/output
    on the sync-engine queue, so activations never queue behind weights.
"""

import sys
from contextlib import ExitStack

if "/opt/trn_rl_repo" not in sys.path:
    sys.path.insert(0, "/opt/trn_rl_repo")

import numpy as np

import concourse.bacc as bacc
import concourse.mybir as mybir
import concourse.tile as tile
from concourse.bass_utils import run_bass_kernel_spmd

B, T, D, NH, HD = 2, 2048, 2048, 16, 128
HPC = 4              # heads per core
PAIR = 256           # queries per group
NPAIR = T // PAIR    # 8
NCHUNK = T // HD     # 16 key chunks of 128
NSLICE = T // PAIR   # 8 t-slices for projections
F32R = mybir.dt.float32r
F32 = mybir.dt.float32
MASK_PRELOAD_MAX = 24


def _mask_structure(mask):
    """Classify each (query-group, key-chunk) block of the additive mask.

    Returns (statuses, maskt): statuses[j] is a list of
    (chunk, mask_tile_index_or_minus1) for blocks that must be computed;
    maskt is the packed (128, nmask, 256) array of deduplicated
    transposed mask tiles for partially-masked blocks.
    """
    statuses = []
    tiles = {}
    tile_list = []
    for j in range(NPAIR):
        q = slice(j * PAIR, (j + 1) * PAIR)
        lst = []
        for c in range(NCHUNK):
            k = slice(c * HD, (c + 1) * HD)
            sub = mask[q, k]
            if np.all(sub <= -1e8):
                continue
            if np.all(sub == 0.0):
                lst.append((c, -1))
            else:
                key = sub.tobytes()
                mi = tiles.get(key)
                if mi is None:
                    mi = len(tile_list)
                    tiles[key] = mi
                    tile_list.append(np.ascontiguousarray(sub.T))
                lst.append((c, mi))
        assert lst, f"query group {j} has every key block masked"
        statuses.append(lst)
    nmask = max(1, len(tile_list))
    maskt = np.zeros((HD, nmask, PAIR), np.float32)
    for i, t in enumerate(tile_list):
        assert np.all(t <= 64.0), "additive mask too large for exp-mask trick"
        maskt[:, i, :] = np.exp(t)
    return statuses, maskt


def _build_program(statuses, nmask):
    nc = bacc.Bacc(None, target_bir_lowering=False)

    xt_d = nc.dram_tensor("xt", [D, T], F32R, kind="ExternalInput")
    wq_d = nc.dram_tensor("wqt", [D, HPC * HD], F32R, kind="ExternalInput")
    wk_d = nc.dram_tensor("wkt", [D, HPC * HD], F32R, kind="ExternalInput")
    wv_d = nc.dram_tensor("wvt", [D, HPC * HD], F32R, kind="ExternalInput")
    wo_d = nc.dram_tensor("wot", [HPC * HD, D], F32R, kind="ExternalInput")
    cs_d = nc.dram_tensor("cs", [HD, 2, T], F32, kind="ExternalInput")
    mk_d = nc.dram_tensor("maskt", [HD, nmask, PAIR], F32, kind="ExternalInput")
    ones_d = nc.dram_tensor("ones_col", [HD, 1], F32R, kind="ExternalInput")
    onesr_d = nc.dram_tensor("ones_row", [1, HD], F32R, kind="ExternalInput")
    out_d = nc.dram_tensor("out", [T, D], F32, kind="ExternalOutput")

    xt_ap = xt_d.ap().rearrange("(k p) t -> p k t", p=HD)
    wq_ap = wq_d.ap().rearrange("(k p) e -> p k e", p=HD)
    wk_ap = wk_d.ap().rearrange("(k p) e -> p k e", p=HD)
    wv_ap = wv_d.ap().rearrange("(k p) e -> p k e", p=HD)
    wo_ap = wo_d.ap().rearrange("(h p) e -> p h e", p=HD)
    EXP = mybir.ActivationFunctionType.Exp
    preload_mask = nmask <= MASK_PRELOAD_MAX

    with tile.TileContext(nc) as tc, ExitStack() as top:
        constp = top.enter_context(tc.tile_pool(name="const", bufs=1))
        ones_sb = constp.tile([HD, 1], F32R)
        onesr_sb = constp.tile([1, HD], F32R)
        nc.scalar.dma_start(ones_sb[:], ones_d[:])
        nc.scalar.dma_start(onesr_sb[:], onesr_d[:])

        qkp = top.enter_context(tc.tile_pool(name="qkp", bufs=1))
        # q heads at [:, h, :], k heads at [:, 4+h, :]
        qk_sb = qkp.tile([HD, 2 * HPC, T], F32R)

        # wv + xt pools span the q/k pass (prefetch) and the v pass
        with ExitStack() as vph:
            wvp = vph.enter_context(tc.tile_pool(name="wvp", side="right", bufs=1))
            wv_sb = wvp.tile([HD, NCHUNK, HPC * HD], F32R)
            xtp = vph.enter_context(tc.tile_pool(name="xtp", side="right", bufs=2))

            # ---- combined q/k projection pass (+ fused RoPE) ----
            with ExitStack() as ph:
                wp = ph.enter_context(tc.tile_pool(name="wp", side="right", bufs=1))
                csp = ph.enter_context(tc.tile_pool(name="csp", side="right", bufs=2))
                ropep = ph.enter_context(tc.tile_pool(name="ropep", side="right", bufs=2))
                pps = ph.enter_context(tc.tile_pool(name="pps", bufs=4, space="PSUM"))
                wqk_sb = wp.tile([HD, 2, NCHUNK, HPC * HD], F32R)
                # weights on the scalar queue, split per k-chunk so the
                # first matmuls start as soon as chunk 0 lands
                for k in range(NCHUNK):
                    nc.scalar.dma_start(wqk_sb[:, 0, k, :], wq_ap[:, k, :])
                for k in range(NCHUNK):
                    nc.scalar.dma_start(wqk_sb[:, 1, k, :], wk_ap[:, k, :])
                for k in range(NCHUNK):  # prefetch wv for the next pass
                    nc.scalar.dma_start(wv_sb[:, k, :], wv_ap[:, k, :])
                work = [(0, 0)]
                for ns in range(1, NSLICE):
                    work += [(ns, 0), (ns - 1, 1)]
                work.append((NSLICE - 1, 1))
                xt_tiles = {}
                cs_tiles = {}
                for ns, wsel in work:
                    tsl = slice(ns * PAIR, (ns + 1) * PAIR)
                    if wsel == 0:
                        xt = xtp.tile([HD, NCHUNK, PAIR], F32R, tag="xt")
                        nc.sync.dma_start(xt[:], xt_ap[:, :, tsl])
                        cs_sl = csp.tile([HD, 2, PAIR], F32, tag="cs")
                        nc.sync.dma_start(cs_sl[:], cs_d[:, :, tsl])
                        xt_tiles[ns], cs_tiles[ns] = xt, cs_sl
                    else:
                        xt, cs_sl = xt_tiles.pop(ns), cs_tiles.pop(ns)
                    if True:
                        for h in range(HPC):
                            ps = pps.tile([HD, PAIR], F32, tag="ps")
                            hs = slice(h * HD, (h + 1) * HD)
                            for k in range(NCHUNK):
                                nc.tensor.matmul(
                                    ps[:],
                                    wqk_sb[:, wsel, k, hs],
                                    xt[:, k, :],
                                    start=(k == 0),
                                    stop=(k == NCHUNK - 1),
                                )
                            # RoPE: dst = raw*C + swap(raw)*S.  The swap is
                            # materialized by two ScalarE half-copies, the S
                            # product runs on GpSimd, so VectorE only does
                            # one multiply and one add per tile.
                            dst = qk_sb[:, wsel * HPC + h, tsl]
                            sw = ropep.tile([HD, PAIR], F32, tag="sw")
                            nc.scalar.copy(sw[0:64, :], ps[64:128, :])
                            nc.scalar.copy(sw[64:128, :], ps[0:64, :])
                            tb = ropep.tile([HD, PAIR], F32R, tag="tb")
                            nc.vector.tensor_mul(dst, ps[:], cs_sl[:, 0, :])
                            nc.vector.tensor_mul(tb[:], sw[:], cs_sl[:, 1, :])
                            nc.vector.tensor_add(dst, dst, tb[:])

            # ---- v projection (normal layout, x^T stationary) ----
            vap = top.enter_context(tc.tile_pool(name="vap", bufs=1))
            v_all = vap.tile([HD, NCHUNK, HPC * HD], F32R)
            with ExitStack() as ph:
                vps = ph.enter_context(tc.tile_pool(name="vps", bufs=4, space="PSUM"))
                for ns in reversed(range(NSLICE)):
                    tsl = slice(ns * PAIR, (ns + 1) * PAIR)
                    xt = xtp.tile([HD, NCHUNK, PAIR], F32R, tag="xt")
                    nc.sync.dma_start(xt[:], xt_ap[:, :, tsl])
                    for tc2 in range(2):
                        ps = vps.tile([HD, HPC * HD], F32, tag="vps")
                        for k in range(NCHUNK):
                            nc.tensor.matmul(
                                ps[:],
                                xt[:, k, tc2 * HD:(tc2 + 1) * HD],
                                wv_sb[:, k, :],
                                start=(k == 0),
                                stop=(k == NCHUNK - 1),
                            )
                        nc.scalar.copy(v_all[:, ns * 2 + tc2, :], ps[:])

        # ---- attention + interleaved output projection ----
        ctxp = top.enter_context(tc.tile_pool(name="ctxp", bufs=1))
        ctx_sb = ctxp.tile([HD, HPC, T], F32R)
        wop = top.enter_context(tc.tile_pool(name="wop", bufs=1))
        wo_sb = wop.tile([HD, HPC, D], F32R)
        with ExitStack() as ph:
            ptp = ph.enter_context(tc.tile_pool(name="ptp", side="right", bufs=2))
            mkp = ph.enter_context(tc.tile_pool(name="mkp", side="right", bufs=4))
            lrp = ph.enter_context(tc.tile_pool(name="lrp", side="right", bufs=2))
            rbp = ph.enter_context(tc.tile_pool(name="rbp", side="right", bufs=2))
            evp = ph.enter_context(tc.tile_pool(name="evp", side="right", bufs=2))
            sps = ph.enter_context(tc.tile_pool(name="sps", bufs=2, space="PSUM"))
            ops = ph.enter_context(tc.tile_pool(name="ops", bufs=2, space="PSUM"))
            lps = ph.enter_context(tc.tile_pool(name="lps", bufs=2, space="PSUM"))

            mk_sb = None
            if preload_mask:
                mkpre = ph.enter_context(
                    tc.tile_pool(name="mkpre", side="right", bufs=1)
                )
                mk_sb = mkpre.tile([HD, nmask, PAIR], F32)
                nc.scalar.dma_start(mk_sb[:], mk_d[:])
            for h in range(HPC):  # prefetch wo
                nc.scalar.dma_start(wo_sb[:, h, :], wo_ap[:, h, :])

            def mask_tile(mi):
                if preload_mask:
                    return mk_sb[:, mi, :]
                mt = mkp.tile([HD, PAIR], F32, tag="mk")
                nc.scalar.dma_start(mt[:], mk_d[:, mi, :])
                return mt[:]

            def finalize(st):
                # off the tensor engine: DVE fast-recip -> GpSimd partition
                # broadcast -> DVE multiply into ctx
                lr = lrp.tile([1, PAIR], F32, tag="lr")
                nc.vector.reciprocal_approx_fast(lr[:], st["l"])
                rb_sb = rbp.tile([HD, PAIR], F32, tag="rb")
                nc.gpsimd.partition_broadcast(rb_sb[:], lr[:])
                nc.vector.tensor_mul(
                    ctx_sb[:, st["h"], st["qsl"]], st["o"], rb_sb[:]
                )

            def emit_ol(dq):
                # deferred p@v and row-sum matmuls for an exp'd quad
                pi, quad, st = dq
                h = st["h"]
                for t, (c, mi) in enumerate(quad):
                    nc.tensor.matmul(
                        st["o"],
                        v_all[:, c, h * HD:(h + 1) * HD],
                        st["pt"][:, pi + t, :],
                        start=(st["oi"] == 0),
                        stop=(st["oi"] == st["n"] - 1),
                        skip_group_check=True,
                    )
                    st["oi"] += 1
                for t, (c, mi) in enumerate(quad):
                    nc.tensor.matmul(
                        st["l"],
                        ones_sb[:],
                        st["pt"][:, pi + t, :],
                        start=(st["li"] == 0),
                        stop=(st["li"] == st["n"] - 1),
                        skip_group_check=True,
                    )
                    st["li"] += 1
                return st["li"] == st["n"]

            def emit_wo(j):
                # output projection for the two t-chunks this pair finished
                for tck in (2 * j, 2 * j + 1):
                    tsl2 = slice(tck * HD, (tck + 1) * HD)
                    for es in range(4):
                        esl = slice(es * 512, (es + 1) * 512)
                        wps = sps.tile([HD, 512], F32, tag="s")
                        for h in range(HPC):
                            nc.tensor.matmul(
                                wps[:],
                                ctx_sb[:, h, tsl2],
                                wo_sb[:, h, esl],
                                start=(h == 0),
                                stop=(h == HPC - 1),
                            )
                        ev = evp.tile([HD, 512], F32, tag="ev")
                        nc.scalar.copy(ev[:], wps[:])
                        nc.sync.dma_start(out_d[tsl2, esl], ev[:])

            pending_ol = None
            pending_fin = None
            wo_ready = None
            for j in reversed(range(NPAIR)):
                qsl = slice(j * PAIR, (j + 1) * PAIR)
                chunks = statuses[j]
                n = len(chunks)
                quads = [chunks[ii:ii + 4] for ii in range(0, n, 4)]
                for h in range(HPC):
                    if h == 2 and wo_ready is not None:
                        emit_wo(wo_ready)
                        wo_ready = None
                    o_ps = ops.tile([HD, PAIR], F32, tag="o")
                    l_ps = lps.tile([1, PAIR], F32, tag="l")
                    pt = ptp.tile([HD, NCHUNK, PAIR], F32R, tag="pt")
                    st = {"o": o_ps[:], "l": l_ps[:],
                          "pt": pt, "h": h, "qsl": qsl, "n": n,
                          "oi": 0, "li": 0}
                    for qi, quad in enumerate(quads):
                        w = len(quad)
                        s_ps = sps.tile([HD, 4, PAIR], F32, tag="s")
                        for t, (c, mi) in enumerate(quad):
                            nc.tensor.matmul(
                                s_ps[:, t, :],
                                qk_sb[:, HPC + h, c * HD:(c + 1) * HD],
                                qk_sb[:, h, qsl],
                                start=True,
                                stop=True,
                            )
                        nc.scalar.activation(
                            pt[:, qi * 4:qi * 4 + w, :], s_ps[:, 0:w, :], EXP
                        )
                        # multiplicative exp-mask applied to pt
                        # (exp(s+m) == exp(s)*exp(m)), off the exp chain
                        t = 0
                        while t < w:
                            c, mi = quad[t]
                            if mi < 0:
                                t += 1
                                continue
                            r = t + 1
                            while (preload_mask and r < w and quad[r][1] >= 0
                                   and quad[r][1] == quad[r - 1][1] + 1):
                                r += 1
                            if preload_mask:
                                sl = slice(qi * 4 + t, qi * 4 + r)
                                nc.vector.tensor_mul(
                                    pt[:, sl, :], pt[:, sl, :],
                                    mk_sb[:, mi:mi + (r - t), :],
                                )
                            else:
                                sl = slice(qi * 4 + t, qi * 4 + t + 1)
                                nc.vector.tensor_mul(
                                    pt[:, sl, :], pt[:, sl, :], mask_tile(mi)
                                )
                                r = t + 1
                            t = r
                        if pending_ol is not None:
                            if emit_ol(pending_ol):
                                pending_fin = pending_ol[2]
                            pending_ol = None
                        if pending_fin is not None and pending_fin is not st:
                            finalize(pending_fin)
                            pending_fin = None
                        pending_ol = (qi * 4, quad, st)
                wo_ready = j
            if pending_ol is not None:
                if emit_ol(pending_ol):
                    pending_fin = pending_ol[2]
            if pending_fin is not None:
                finalize(pending_fin)
            if wo_ready is not None:
                emit_wo(wo_ready)
    nc.compile()
    return nc


_PERM = np.concatenate(
    [np.concatenate([np.arange(0, HD, 2), np.arange(1, HD, 2)]) + h * HD
     for h in range(HPC)]
)


def prepare(x, freqs, mask, wq, wk, wv, wo):
    """Host-side sharding/prep. Returns (nc, in_maps)."""
    x = np.asarray(x, np.float32)
    freqs = np.asarray(freqs, np.float32)
    mask = np.asarray(mask, np.float32)
    wq, wk, wv, wo = (np.asarray(w, np.float32) for w in (wq, wk, wv, wo))

    statuses, maskt = _mask_structure(mask)
    nc = _build_program(statuses, maskt.shape[1])

    scale = np.float32(1.0 / np.sqrt(HD))
    cos = np.ascontiguousarray(freqs[:, :, 0].T)  # (64, T)
    sin = np.ascontiguousarray(freqs[:, :, 1].T)
    cs = np.empty((HD, 2, T), np.float32)
    cs[0:64, 0, :] = cos
    cs[64:128, 0, :] = cos
    cs[0:64, 1, :] = -sin
    cs[64:128, 1, :] = sin

    ones_col = np.ones((HD, 1), np.float32)
    ones_row = np.ones((1, HD), np.float32)
    xt = [np.ascontiguousarray(x[b].T) for b in range(B)]

    in_maps = []
    for core in range(8):
        b, g = core // 4, core % 4
        cols = slice(g * HPC * HD, (g + 1) * HPC * HD)
        in_maps.append({
            "xt": xt[b],
            "wqt": np.ascontiguousarray((wq.T[:, cols] * scale)[:, _PERM]),
            "wkt": np.ascontiguousarray(wk.T[:, cols][:, _PERM]),
            "wvt": np.ascontiguousarray(wv.T[:, cols]),
            "wot": np.ascontiguousarray(wo.T[cols, :]),
            "cs": cs,
            "maskt": maskt,
            "ones_col": ones_col,
            "ones_row": ones_row,
        })
    return nc, in_maps


def run(x, freqs, mask, wq, wk, wv, wo, **spmd_kwargs):
    nc, in_maps = prepare(x, freqs, mask, wq, wk, wv, wo)
    res = run_bass_kernel_spmd(nc, in_maps, list(range(8)), **spmd_kwargs)
    parts = [res.results[c]["out"] for c in range(8)]
    out = np.stack([
        parts[b * 4] + parts[b * 4 + 1] + parts[b * 4 + 2] + parts[b * 4 + 3]
        for b in range(B)
    ]).astype(np.float32)
    return out, res


def kernel(x, freqs, mask, wq, wk, wv, wo):
    out, _ = run(x, freqs, mask, wq, wk, wv, wo)
    return out


# revision 14
# speedup vs baseline: 1.1036x; 1.1036x over previous
"""Trainium2 Bass kernel for causal self-attention with RoPE.

Shapes: x (2, 2048, 2048), 16 heads x 128 head_dim.
Sharding: 8 cores = 2 batch x 4 head-groups (4 heads per core).
Each core computes q/k/v projections for its heads, RoPE, causal-masked
softmax attention, and a partial output projection (its head columns of
wo); the host sums the 4 partials per batch element.

Layout strategy (per core):
  - q,k built in transposed layout (head_dim on partitions, t free) so
    RoPE and the score matmuls need no on-device transposes.  The host
    permutes wq/wk columns so RoPE's even/odd pairs become the two
    partition halves, and pre-scales wq by 1/sqrt(head_dim).
  - scores computed as s^T (keys x q) per 256-query group; softmax skips
    the max-subtraction (scores are O(1) by construction); row sums via
    an ones-vector matmul; normalization folded into the PSUM eviction.
  - v computed directly in (t x e) layout by using x^T as the stationary
    operand, so the p@v matmul needs no transposes anywhere.
  - fully-masked key blocks are skipped (host inspects the mask), which
    halves the attention work for the causal mask; deduplicated mask
    tiles are added only where a block is partially masked.
  - matmuls run in float32r (tf32) which streams at full rate for moving
    dims >= 256.
  - weights/mask stream on the scalar-engine DMA queue, x^T/tables/output
    on the sync-engine queue, so activations never queue behind weights.
"""

import sys
from contextlib import ExitStack

if "/opt/trn_rl_repo" not in sys.path:
    sys.path.insert(0, "/opt/trn_rl_repo")

import numpy as np

import concourse.bacc as bacc
import concourse.mybir as mybir
import concourse.tile as tile
from concourse.bass_utils import run_bass_kernel_spmd

B, T, D, NH, HD = 2, 2048, 2048, 16, 128
HPC = 4              # heads per core
PAIR = 256           # queries per group
NPAIR = T // PAIR    # 8
NCHUNK = T // HD     # 16 key chunks of 128
NSLICE = T // PAIR   # 8 t-slices for projections
F32R = mybir.dt.float32r
F32 = mybir.dt.float32
MASK_PRELOAD_MAX = 24


def _mask_structure(mask):
    """Classify each (query-group, key-chunk) block of the additive mask.

    Returns (statuses, maskt): statuses[j] is a list of
    (chunk, mask_tile_index_or_minus1) for blocks that must be computed;
    maskt is the packed (128, nmask, 256) array of deduplicated
    transposed mask tiles for partially-masked blocks.
    """
    statuses = []
    tiles = {}
    tile_list = []
    for j in range(NPAIR):
        q = slice(j * PAIR, (j + 1) * PAIR)
        lst = []
        for c in range(NCHUNK):
            k = slice(c * HD, (c + 1) * HD)
            sub = mask[q, k]
            if np.all(sub <= -1e8):
                continue
            if np.all(sub == 0.0):
                lst.append((c, -1))
            else:
                key = sub.tobytes()
                mi = tiles.get(key)
                if mi is None:
                    mi = len(tile_list)
                    tiles[key] = mi
                    tile_list.append(np.ascontiguousarray(sub.T))
                lst.append((c, mi))
        assert lst, f"query group {j} has every key block masked"
        statuses.append(lst)
    nmask = max(1, len(tile_list))
    maskt = np.zeros((HD, nmask, PAIR), np.float32)
    for i, t in enumerate(tile_list):
        assert np.all(t <= 64.0), "additive mask too large for exp-mask trick"
        maskt[:, i, :] = np.exp(t)
    return statuses, maskt


def _build_program(statuses, nmask):
    nc = bacc.Bacc(None, target_bir_lowering=False)

    xt_d = nc.dram_tensor("xt", [D, T], F32R, kind="ExternalInput")
    wq_d = nc.dram_tensor("wqt", [D, HPC * HD], F32R, kind="ExternalInput")
    wk_d = nc.dram_tensor("wkt", [D, HPC * HD], F32R, kind="ExternalInput")
    wv_d = nc.dram_tensor("wvt", [D, HPC * HD], F32R, kind="ExternalInput")
    wo_d = nc.dram_tensor("wot", [HPC * HD, D], F32R, kind="ExternalInput")
    cs_d = nc.dram_tensor("cs", [HD, 2, T], F32, kind="ExternalInput")
    mk_d = nc.dram_tensor("maskt", [HD, nmask, PAIR], F32, kind="ExternalInput")
    ones_d = nc.dram_tensor("ones_col", [HD, 1], F32R, kind="ExternalInput")
    onesr_d = nc.dram_tensor("ones_row", [1, HD], F32R, kind="ExternalInput")
    out_d = nc.dram_tensor("out", [T, D], F32, kind="ExternalOutput")

    xt_ap = xt_d.ap().rearrange("(k p) t -> p k t", p=HD)
    wq_ap = wq_d.ap().rearrange("(k p) e -> p k e", p=HD)
    wk_ap = wk_d.ap().rearrange("(k p) e -> p k e", p=HD)
    wv_ap = wv_d.ap().rearrange("(k p) e -> p k e", p=HD)
    wo_ap = wo_d.ap().rearrange("(h p) e -> p h e", p=HD)
    EXP = mybir.ActivationFunctionType.Exp
    preload_mask = nmask <= MASK_PRELOAD_MAX

    with tile.TileContext(nc) as tc, ExitStack() as top:
        constp = top.enter_context(tc.tile_pool(name="const", bufs=1))
        ones_sb = constp.tile([HD, 1], F32R)
        onesr_sb = constp.tile([1, HD], F32R)
        nc.scalar.dma_start(ones_sb[:], ones_d[:])
        nc.scalar.dma_start(onesr_sb[:], onesr_d[:])

        qkp = top.enter_context(tc.tile_pool(name="qkp", bufs=1))
        # q heads at [:, h, :], k heads at [:, 4+h, :]
        qk_sb = qkp.tile([HD, 2 * HPC, T], F32R)

        # wv + xt pools span the q/k pass (prefetch) and the v pass
        with ExitStack() as vph:
            wvp = vph.enter_context(tc.tile_pool(name="wvp", side="right", bufs=1))
            wv_sb = wvp.tile([HD, NCHUNK, HPC * HD], F32R)
            xtp = vph.enter_context(tc.tile_pool(name="xtp", side="right", bufs=2))

            # ---- combined q/k projection pass (+ fused RoPE) ----
            with ExitStack() as ph:
                wp = ph.enter_context(tc.tile_pool(name="wp", side="right", bufs=1))
                csp = ph.enter_context(tc.tile_pool(name="csp", side="right", bufs=2))
                ropep = ph.enter_context(tc.tile_pool(name="ropep", side="right", bufs=2))
                pps = ph.enter_context(tc.tile_pool(name="pps", bufs=4, space="PSUM"))
                wqk_sb = wp.tile([HD, 2, NCHUNK, HPC * HD], F32R)
                # weights on the scalar queue, split per k-chunk so the
                # first matmuls start as soon as chunk 0 lands
                for k in range(NCHUNK):
                    nc.scalar.dma_start(wqk_sb[:, 0, k, :], wq_ap[:, k, :])
                for k in range(NCHUNK):
                    nc.scalar.dma_start(wqk_sb[:, 1, k, :], wk_ap[:, k, :])
                for k in range(NCHUNK):  # prefetch wv for the next pass
                    nc.scalar.dma_start(wv_sb[:, k, :], wv_ap[:, k, :])
                work = [(0, 0)]
                for ns in range(1, NSLICE):
                    work += [(ns, 0), (ns - 1, 1)]
                work.append((NSLICE - 1, 1))
                xt_tiles = {}
                cs_tiles = {}
                for ns, wsel in work:
                    tsl = slice(ns * PAIR, (ns + 1) * PAIR)
                    if wsel == 0:
                        xt = xtp.tile([HD, NCHUNK, PAIR], F32R, tag="xt")
                        nc.sync.dma_start(xt[:], xt_ap[:, :, tsl])
                        cs_sl = csp.tile([HD, 2, PAIR], F32, tag="cs")
                        nc.sync.dma_start(cs_sl[:], cs_d[:, :, tsl])
                        xt_tiles[ns], cs_tiles[ns] = xt, cs_sl
                    else:
                        xt, cs_sl = xt_tiles.pop(ns), cs_tiles.pop(ns)
                    if True:
                        for h in range(HPC):
                            ps = pps.tile([HD, PAIR], F32, tag="ps")
                            hs = slice(h * HD, (h + 1) * HD)
                            for k in range(NCHUNK):
                                nc.tensor.matmul(
                                    ps[:],
                                    wqk_sb[:, wsel, k, hs],
                                    xt[:, k, :],
                                    start=(k == 0),
                                    stop=(k == NCHUNK - 1),
                                )
                            # RoPE: dst = raw*C + swap(raw)*S.  The swap is
                            # materialized by two ScalarE half-copies, the S
                            # product runs on GpSimd, so VectorE only does
                            # one multiply and one add per tile.
                            dst = qk_sb[:, wsel * HPC + h, tsl]
                            sw = ropep.tile([HD, PAIR], F32, tag="sw")
                            nc.scalar.copy(sw[0:64, :], ps[64:128, :])
                            nc.scalar.copy(sw[64:128, :], ps[0:64, :])
                            tb = ropep.tile([HD, PAIR], F32R, tag="tb")
                            nc.vector.tensor_mul(dst, ps[:], cs_sl[:, 0, :])
                            nc.vector.tensor_mul(tb[:], sw[:], cs_sl[:, 1, :])
                            nc.vector.tensor_add(dst, dst, tb[:])

            # ---- v projection (normal layout, x^T stationary) ----
            vap = top.enter_context(tc.tile_pool(name="vap", bufs=1))
            v_all = vap.tile([HD, NCHUNK, HPC * HD], F32R)
            with ExitStack() as ph:
                vps = ph.enter_context(tc.tile_pool(name="vps", bufs=4, space="PSUM"))
                for ns in reversed(range(NSLICE)):
                    tsl = slice(ns * PAIR, (ns + 1) * PAIR)
                    xt = xtp.tile([HD, NCHUNK, PAIR], F32R, tag="xt")
                    nc.sync.dma_start(xt[:], xt_ap[:, :, tsl])
                    for tc2 in range(2):
                        ps = vps.tile([HD, HPC * HD], F32, tag="vps")
                        for k in range(NCHUNK):
                            nc.tensor.matmul(
                                ps[:],
                                xt[:, k, tc2 * HD:(tc2 + 1) * HD],
                                wv_sb[:, k, :],
                                start=(k == 0),
                                stop=(k == NCHUNK - 1),
                            )
                        nc.scalar.copy(v_all[:, ns * 2 + tc2, :], ps[:])

        # ---- attention + interleaved output projection ----
        ctxp = top.enter_context(tc.tile_pool(name="ctxp", bufs=1))
        ctx_sb = ctxp.tile([HD, HPC, T], F32R)
        wop = top.enter_context(tc.tile_pool(name="wop", bufs=1))
        wo_sb = wop.tile([HD, HPC, D], F32R)
        with ExitStack() as ph:
            ptp = ph.enter_context(tc.tile_pool(name="ptp", side="right", bufs=2))
            mkp = ph.enter_context(tc.tile_pool(name="mkp", side="right", bufs=4))
            lrp = ph.enter_context(tc.tile_pool(name="lrp", side="right", bufs=2))
            rbp = ph.enter_context(tc.tile_pool(name="rbp", side="right", bufs=2))
            sps = ph.enter_context(tc.tile_pool(name="sps", bufs=2, space="PSUM"))
            ops = ph.enter_context(tc.tile_pool(name="ops", bufs=2, space="PSUM"))
            lps = ph.enter_context(tc.tile_pool(name="lps", bufs=2, space="PSUM"))

            mk_sb = None
            if preload_mask:
                mkpre = ph.enter_context(
                    tc.tile_pool(name="mkpre", side="right", bufs=1)
                )
                mk_sb = mkpre.tile([HD, nmask, PAIR], F32)
                nc.scalar.dma_start(mk_sb[:], mk_d[:])
            for h in range(HPC):  # prefetch wo
                nc.scalar.dma_start(wo_sb[:, h, :], wo_ap[:, h, :])

            def mask_tile(mi):
                if preload_mask:
                    return mk_sb[:, mi, :]
                mt = mkp.tile([HD, PAIR], F32, tag="mk")
                nc.scalar.dma_start(mt[:], mk_d[:, mi, :])
                return mt[:]

            def finalize(st):
                # off the tensor engine: DVE fast-recip -> GpSimd partition
                # broadcast -> DVE multiply into ctx
                lr = lrp.tile([1, PAIR], F32, tag="lr")
                nc.vector.reciprocal_approx_fast(lr[:], st["l"])
                rb_sb = rbp.tile([HD, PAIR], F32, tag="rb")
                nc.gpsimd.partition_broadcast(rb_sb[:], lr[:])
                nc.vector.tensor_mul(
                    ctx_sb[:, st["h"], st["qsl"]], st["o"], rb_sb[:]
                )

            def emit_ol(dq):
                # deferred p@v and row-sum matmuls for an exp'd quad
                pi, quad, st = dq
                h = st["h"]
                for t, (c, mi) in enumerate(quad):
                    nc.tensor.matmul(
                        st["o"],
                        v_all[:, c, h * HD:(h + 1) * HD],
                        st["pt"][:, pi + t, :],
                        start=(st["oi"] == 0),
                        stop=(st["oi"] == st["n"] - 1),
                        skip_group_check=True,
                    )
                    st["oi"] += 1
                for t, (c, mi) in enumerate(quad):
                    nc.tensor.matmul(
                        st["l"],
                        ones_sb[:],
                        st["pt"][:, pi + t, :],
                        start=(st["li"] == 0),
                        stop=(st["li"] == st["n"] - 1),
                        skip_group_check=True,
                    )
                    st["li"] += 1
                return st["li"] == st["n"]

            pending_ol = None
            pending_fin = None
            for j in reversed(range(NPAIR)):
                qsl = slice(j * PAIR, (j + 1) * PAIR)
                chunks = statuses[j]
                n = len(chunks)
                quads = [chunks[ii:ii + 4] for ii in range(0, n, 4)]
                for h in range(HPC):
                    o_ps = ops.tile([HD, PAIR], F32, tag="o")
                    l_ps = lps.tile([1, PAIR], F32, tag="l")
                    pt = ptp.tile([HD, NCHUNK, PAIR], F32R, tag="pt")
                    st = {"o": o_ps[:], "l": l_ps[:],
                          "pt": pt, "h": h, "qsl": qsl, "n": n,
                          "oi": 0, "li": 0}
                    for qi, quad in enumerate(quads):
                        w = len(quad)
                        s_ps = sps.tile([HD, 4, PAIR], F32, tag="s")
                        for t, (c, mi) in enumerate(quad):
                            nc.tensor.matmul(
                                s_ps[:, t, :],
                                qk_sb[:, HPC + h, c * HD:(c + 1) * HD],
                                qk_sb[:, h, qsl],
                                start=True,
                                stop=True,
                            )
                        nc.scalar.activation(
                            pt[:, qi * 4:qi * 4 + w, :], s_ps[:, 0:w, :], EXP
                        )
                        # multiplicative exp-mask applied to pt
                        # (exp(s+m) == exp(s)*exp(m)), off the exp chain
                        t = 0
                        while t < w:
                            c, mi = quad[t]
                            if mi < 0:
                                t += 1
                                continue
                            r = t + 1
                            while (preload_mask and r < w and quad[r][1] >= 0
                                   and quad[r][1] == quad[r - 1][1] + 1):
                                r += 1
                            if preload_mask:
                                sl = slice(qi * 4 + t, qi * 4 + r)
                                nc.vector.tensor_mul(
                                    pt[:, sl, :], pt[:, sl, :],
                                    mk_sb[:, mi:mi + (r - t), :],
                                )
                            else:
                                sl = slice(qi * 4 + t, qi * 4 + t + 1)
                                nc.vector.tensor_mul(
                                    pt[:, sl, :], pt[:, sl, :], mask_tile(mi)
                                )
                                r = t + 1
                            t = r
                        if pending_ol is not None:
                            if emit_ol(pending_ol):
                                pending_fin = pending_ol[2]
                            pending_ol = None
                        if pending_fin is not None and pending_fin is not st:
                            finalize(pending_fin)
                            pending_fin = None
                        pending_ol = (qi * 4, quad, st)
            if pending_ol is not None:
                if emit_ol(pending_ol):
                    pending_fin = pending_ol[2]
            if pending_fin is not None:
                finalize(pending_fin)
        # ---- output projection ----
        with ExitStack() as ph:
            evp = ph.enter_context(tc.tile_pool(name="evp", side="right", bufs=4))
            wops = ph.enter_context(tc.tile_pool(name="wops", bufs=4, space="PSUM"))
            for tck in range(NCHUNK):
                tsl = slice(tck * HD, (tck + 1) * HD)
                for es in range(4):
                    esl = slice(es * 512, (es + 1) * 512)
                    ps = wops.tile([HD, 512], F32, tag="wo")
                    for h in range(HPC):
                        nc.tensor.matmul(
                            ps[:],
                            ctx_sb[:, h, tsl],
                            wo_sb[:, h, esl],
                            start=(h == 0),
                            stop=(h == HPC - 1),
                        )
                    ev = evp.tile([HD, 512], F32, tag="ev")
                    nc.scalar.copy(ev[:], ps[:])
                    nc.sync.dma_start(out_d[tsl, esl], ev[:])
    nc.compile()
    return nc


_PERM = np.concatenate(
    [np.concatenate([np.arange(0, HD, 2), np.arange(1, HD, 2)]) + h * HD
     for h in range(HPC)]
)


def prepare(x, freqs, mask, wq, wk, wv, wo):
    """Host-side sharding/prep. Returns (nc, in_maps)."""
    x = np.asarray(x, np.float32)
    freqs = np.asarray(freqs, np.float32)
    mask = np.asarray(mask, np.float32)
    wq, wk, wv, wo = (np.asarray(w, np.float32) for w in (wq, wk, wv, wo))

    statuses, maskt = _mask_structure(mask)
    nc = _build_program(statuses, maskt.shape[1])

    scale = np.float32(1.0 / np.sqrt(HD))
    cos = np.ascontiguousarray(freqs[:, :, 0].T)  # (64, T)
    sin = np.ascontiguousarray(freqs[:, :, 1].T)
    cs = np.empty((HD, 2, T), np.float32)
    cs[0:64, 0, :] = cos
    cs[64:128, 0, :] = cos
    cs[0:64, 1, :] = -sin
    cs[64:128, 1, :] = sin

    ones_col = np.ones((HD, 1), np.float32)
    ones_row = np.ones((1, HD), np.float32)
    xt = [np.ascontiguousarray(x[b].T) for b in range(B)]

    in_maps = []
    for core in range(8):
        b, g = core // 4, core % 4
        cols = slice(g * HPC * HD, (g + 1) * HPC * HD)
        in_maps.append({
            "xt": xt[b],
            "wqt": np.ascontiguousarray((wq.T[:, cols] * scale)[:, _PERM]),
            "wkt": np.ascontiguousarray(wk.T[:, cols][:, _PERM]),
            "wvt": np.ascontiguousarray(wv.T[:, cols]),
            "wot": np.ascontiguousarray(wo.T[cols, :]),
            "cs": cs,
            "maskt": maskt,
            "ones_col": ones_col,
            "ones_row": ones_row,
        })
    return nc, in_maps


def run(x, freqs, mask, wq, wk, wv, wo, **spmd_kwargs):
    nc, in_maps = prepare(x, freqs, mask, wq, wk, wv, wo)
    res = run_bass_kernel_spmd(nc, in_maps, list(range(8)), **spmd_kwargs)
    parts = [res.results[c]["out"] for c in range(8)]
    out = np.stack([
        parts[b * 4] + parts[b * 4 + 1] + parts[b * 4 + 2] + parts[b * 4 + 3]
        for b in range(B)
    ]).astype(np.float32)
    return out, res


def kernel(x, freqs, mask, wq, wk, wv, wo):
    out, _ = run(x, freqs, mask, wq, wk, wv, wo)
    return out


# revision 15
# speedup vs baseline: 1.2611x; 1.1427x over previous
"""Trainium2 Bass kernel for causal self-attention with RoPE.

Shapes: x (2, 2048, 2048), 16 heads x 128 head_dim.
Sharding: 8 cores = 2 batch x 4 head-groups (4 heads per core).
Each core computes q/k/v projections for its heads, RoPE, causal-masked
softmax attention, and a partial output projection (its head columns of
wo); the host sums the 4 partials per batch element.

Layout strategy (per core):
  - q,k built in transposed layout (head_dim on partitions, t free) so
    RoPE and the score matmuls need no on-device transposes.  The host
    permutes wq/wk columns so RoPE's even/odd pairs become the two
    partition halves, and pre-scales wq by 1/sqrt(head_dim).
  - scores computed as s^T (keys x q) per 256-query group; softmax skips
    the max-subtraction (scores are O(1) by construction); row sums via
    an ones-vector matmul; normalization folded into the PSUM eviction.
  - v computed directly in (t x e) layout by using x^T as the stationary
    operand, so the p@v matmul needs no transposes anywhere.
  - fully-masked key blocks are skipped (host inspects the mask), which
    halves the attention work for the causal mask; deduplicated mask
    tiles are added only where a block is partially masked.
  - matmuls run in float32r (tf32) which streams at full rate for moving
    dims >= 256.
  - weights/mask stream on the scalar-engine DMA queue, x^T/tables/output
    on the sync-engine queue, so activations never queue behind weights.
"""

import sys
from contextlib import ExitStack

if "/opt/trn_rl_repo" not in sys.path:
    sys.path.insert(0, "/opt/trn_rl_repo")

import numpy as np

import concourse.bacc as bacc
import concourse.mybir as mybir
import concourse.tile as tile
from concourse.bass_utils import run_bass_kernel_spmd

B, T, D, NH, HD = 2, 2048, 2048, 16, 128
HPC = 4              # heads per core
PAIR = 256           # queries per group
NPAIR = T // PAIR    # 8
NCHUNK = T // HD     # 16 key chunks of 128
NSLICE = T // PAIR   # 8 t-slices for projections
F32R = mybir.dt.float32r
F32 = mybir.dt.float32
MASK_PRELOAD_MAX = 24


def _mask_structure(mask):
    """Classify each (query-group, key-chunk) block of the additive mask.

    Returns (statuses, maskt): statuses[j] is a list of
    (chunk, mask_tile_index_or_minus1) for blocks that must be computed;
    maskt is the packed (128, nmask, 256) array of deduplicated
    transposed mask tiles for partially-masked blocks.
    """
    statuses = []
    tiles = {}
    tile_list = []
    for j in range(NPAIR):
        q = slice(j * PAIR, (j + 1) * PAIR)
        lst = []
        for c in range(NCHUNK):
            k = slice(c * HD, (c + 1) * HD)
            sub = mask[q, k]
            if np.all(sub <= -1e8):
                continue
            if np.all(sub == 0.0):
                lst.append((c, -1))
            else:
                key = sub.tobytes()
                mi = tiles.get(key)
                if mi is None:
                    mi = len(tile_list)
                    tiles[key] = mi
                    tile_list.append(np.ascontiguousarray(sub.T))
                lst.append((c, mi))
        assert lst, f"query group {j} has every key block masked"
        statuses.append(lst)
    nmask = max(1, len(tile_list))
    maskt = np.zeros((HD, nmask, PAIR), np.float32)
    for i, t in enumerate(tile_list):
        assert np.all(t <= 64.0), "additive mask too large for exp-mask trick"
        maskt[:, i, :] = np.exp(t)
    return statuses, maskt


def _build_program(statuses, nmask):
    nc = bacc.Bacc(None, target_bir_lowering=False)

    xt_d = nc.dram_tensor("xt", [D, T], F32R, kind="ExternalInput")
    wq_d = nc.dram_tensor("wqt", [D, HPC * HD], F32R, kind="ExternalInput")
    wk_d = nc.dram_tensor("wkt", [D, HPC * HD], F32R, kind="ExternalInput")
    wv_d = nc.dram_tensor("wvt", [D, HPC * HD], F32R, kind="ExternalInput")
    wo_d = nc.dram_tensor("wot", [HPC * HD, D], F32R, kind="ExternalInput")
    cs_d = nc.dram_tensor("cs", [HD, 2, T], F32, kind="ExternalInput")
    mk_d = nc.dram_tensor("maskt", [HD, nmask, PAIR], F32, kind="ExternalInput")
    ones_d = nc.dram_tensor("ones_col", [HD, 1], F32R, kind="ExternalInput")
    onesr_d = nc.dram_tensor("ones_row", [1, HD], F32R, kind="ExternalInput")
    out_d = nc.dram_tensor("out", [T, D], F32, kind="ExternalOutput")

    xt_ap = xt_d.ap().rearrange("(k p) t -> p k t", p=HD)
    wq_ap = wq_d.ap().rearrange("(k p) e -> p k e", p=HD)
    wk_ap = wk_d.ap().rearrange("(k p) e -> p k e", p=HD)
    wv_ap = wv_d.ap().rearrange("(k p) e -> p k e", p=HD)
    wo_ap = wo_d.ap().rearrange("(h p) e -> p h e", p=HD)
    EXP = mybir.ActivationFunctionType.Exp
    preload_mask = nmask <= MASK_PRELOAD_MAX

    with tile.TileContext(nc) as tc, ExitStack() as top:
        constp = top.enter_context(tc.tile_pool(name="const", bufs=1))
        ones_sb = constp.tile([HD, 1], F32R)
        onesr_sb = constp.tile([1, HD], F32R)
        nc.scalar.dma_start(ones_sb[:], ones_d[:])
        nc.scalar.dma_start(onesr_sb[:], onesr_d[:])

        qkp = top.enter_context(tc.tile_pool(name="qkp", bufs=1))
        # q heads at [:, h, :], k heads at [:, 4+h, :]
        qk_sb = qkp.tile([HD, 2 * HPC, T], F32R)

        # wv + xt pools span the q/k pass (prefetch) and the v pass
        with ExitStack() as vph:
            wvp = vph.enter_context(tc.tile_pool(name="wvp", side="right", bufs=1))
            wv_sb = wvp.tile([HD, NCHUNK, HPC * HD], F32R)
            xtp = vph.enter_context(tc.tile_pool(name="xtp", side="right", bufs=2))

            # ---- combined q/k projection pass (+ fused RoPE) ----
            with ExitStack() as ph:
                wp = ph.enter_context(tc.tile_pool(name="wp", side="right", bufs=1))
                csp = ph.enter_context(tc.tile_pool(name="csp", side="right", bufs=2))
                ropep = ph.enter_context(tc.tile_pool(name="ropep", side="right", bufs=2))
                pps = ph.enter_context(tc.tile_pool(name="pps", bufs=4, space="PSUM"))
                wqk_sb = wp.tile([HD, 2, NCHUNK, HPC * HD], F32R)
                # weights on the scalar queue, split per k-chunk so the
                # first matmuls start as soon as chunk 0 lands
                for k in range(NCHUNK):
                    nc.scalar.dma_start(wqk_sb[:, 0, k, :], wq_ap[:, k, :])
                for k in range(NCHUNK):
                    nc.scalar.dma_start(wqk_sb[:, 1, k, :], wk_ap[:, k, :])
                for k in range(NCHUNK):  # prefetch wv for the next pass
                    nc.scalar.dma_start(wv_sb[:, k, :], wv_ap[:, k, :])
                for ns in range(NSLICE):
                    tsl = slice(ns * PAIR, (ns + 1) * PAIR)
                    xt = xtp.tile([HD, NCHUNK, PAIR], F32R, tag="xt")
                    nc.sync.dma_start(xt[:], xt_ap[:, :, tsl])
                    cs_sl = csp.tile([HD, 2, PAIR], F32, tag="cs")
                    nc.sync.dma_start(cs_sl[:], cs_d[:, :, tsl])
                    for wsel in range(2):
                        for h in range(HPC):
                            ps = pps.tile([HD, PAIR], F32, tag="ps")
                            hs = slice(h * HD, (h + 1) * HD)
                            for k in range(NCHUNK):
                                nc.tensor.matmul(
                                    ps[:],
                                    wqk_sb[:, wsel, k, hs],
                                    xt[:, k, :],
                                    start=(k == 0),
                                    stop=(k == NCHUNK - 1),
                                )
                            # RoPE: dst = raw*C + swap(raw)*S.  The swap is
                            # materialized by two ScalarE half-copies, the S
                            # product runs on GpSimd, so VectorE only does
                            # one multiply and one add per tile.
                            dst = qk_sb[:, wsel * HPC + h, tsl]
                            sw = ropep.tile([HD, PAIR], F32, tag="sw")
                            nc.scalar.copy(sw[0:64, :], ps[64:128, :])
                            nc.scalar.copy(sw[64:128, :], ps[0:64, :])
                            tb = ropep.tile([HD, PAIR], F32R, tag="tb")
                            nc.vector.tensor_mul(dst, ps[:], cs_sl[:, 0, :])
                            nc.vector.tensor_mul(tb[:], sw[:], cs_sl[:, 1, :])
                            nc.vector.tensor_add(dst, dst, tb[:])

            # ---- v projection (normal layout, x^T stationary) ----
            vap = top.enter_context(tc.tile_pool(name="vap", bufs=1))
            v_all = vap.tile([HD, NCHUNK, HPC * HD], F32R)
            with ExitStack() as ph:
                vps = ph.enter_context(tc.tile_pool(name="vps", bufs=4, space="PSUM"))
                for ns in reversed(range(NSLICE)):
                    tsl = slice(ns * PAIR, (ns + 1) * PAIR)
                    xt = xtp.tile([HD, NCHUNK, PAIR], F32R, tag="xt")
                    nc.sync.dma_start(xt[:], xt_ap[:, :, tsl])
                    for tc2 in range(2):
                        ps = vps.tile([HD, HPC * HD], F32, tag="vps")
                        for k in range(NCHUNK):
                            nc.tensor.matmul(
                                ps[:],
                                xt[:, k, tc2 * HD:(tc2 + 1) * HD],
                                wv_sb[:, k, :],
                                start=(k == 0),
                                stop=(k == NCHUNK - 1),
                            )
                        nc.scalar.copy(v_all[:, ns * 2 + tc2, :], ps[:])

        # ---- attention + interleaved output projection ----
        ctxp = top.enter_context(tc.tile_pool(name="ctxp", bufs=1))
        ctx_sb = ctxp.tile([HD, HPC, T], F32R)
        wop = top.enter_context(tc.tile_pool(name="wop", bufs=1))
        wo_sb = wop.tile([HD, HPC, D], F32R)
        with ExitStack() as ph:
            ptp = ph.enter_context(tc.tile_pool(name="ptp", side="right", bufs=2))
            mkp = ph.enter_context(tc.tile_pool(name="mkp", side="right", bufs=4))
            lrp = ph.enter_context(tc.tile_pool(name="lrp", side="right", bufs=2))
            rbp = ph.enter_context(tc.tile_pool(name="rbp", side="right", bufs=2))
            sps = ph.enter_context(tc.tile_pool(name="sps", bufs=2, space="PSUM"))
            ops = ph.enter_context(tc.tile_pool(name="ops", bufs=2, space="PSUM"))
            lps = ph.enter_context(tc.tile_pool(name="lps", bufs=2, space="PSUM"))

            mk_sb = None
            if preload_mask:
                mkpre = ph.enter_context(
                    tc.tile_pool(name="mkpre", side="right", bufs=1)
                )
                mk_sb = mkpre.tile([HD, nmask, PAIR], F32)
                nc.scalar.dma_start(mk_sb[:], mk_d[:])
            for h in range(HPC):  # prefetch wo
                nc.scalar.dma_start(wo_sb[:, h, :], wo_ap[:, h, :])

            def mask_tile(mi):
                if preload_mask:
                    return mk_sb[:, mi, :]
                mt = mkp.tile([HD, PAIR], F32, tag="mk")
                nc.scalar.dma_start(mt[:], mk_d[:, mi, :])
                return mt[:]

            def finalize(st):
                # off the tensor engine: DVE fast-recip -> GpSimd partition
                # broadcast -> DVE multiply into ctx
                lr = lrp.tile([1, PAIR], F32, tag="lr")
                nc.vector.reciprocal_approx_fast(lr[:], st["l"])
                rb_sb = rbp.tile([HD, PAIR], F32, tag="rb")
                nc.gpsimd.partition_broadcast(rb_sb[:], lr[:])
                nc.vector.tensor_mul(
                    ctx_sb[:, st["h"], st["qsl"]], st["o"], rb_sb[:]
                )

            def emit_ol(dq):
                # deferred p@v and row-sum matmuls for an exp'd quad
                pi, quad, st = dq
                h = st["h"]
                for t, (c, mi) in enumerate(quad):
                    nc.tensor.matmul(
                        st["o"],
                        v_all[:, c, h * HD:(h + 1) * HD],
                        st["pt"][:, pi + t, :],
                        start=(st["oi"] == 0),
                        stop=(st["oi"] == st["n"] - 1),
                        skip_group_check=True,
                    )
                    st["oi"] += 1
                for t, (c, mi) in enumerate(quad):
                    nc.tensor.matmul(
                        st["l"],
                        ones_sb[:],
                        st["pt"][:, pi + t, :],
                        start=(st["li"] == 0),
                        stop=(st["li"] == st["n"] - 1),
                        skip_group_check=True,
                    )
                    st["li"] += 1
                return st["li"] == st["n"]

            pending_ol = None
            pending_fin = None
            for j in reversed(range(NPAIR)):
                qsl = slice(j * PAIR, (j + 1) * PAIR)
                chunks = statuses[j]
                n = len(chunks)
                quads = [chunks[ii:ii + 4] for ii in range(0, n, 4)]
                for h in range(HPC):
                    o_ps = ops.tile([HD, PAIR], F32, tag="o")
                    l_ps = lps.tile([1, PAIR], F32, tag="l")
                    pt = ptp.tile([HD, NCHUNK, PAIR], F32R, tag="pt")
                    st = {"o": o_ps[:], "l": l_ps[:],
                          "pt": pt, "h": h, "qsl": qsl, "n": n,
                          "oi": 0, "li": 0}
                    for qi, quad in enumerate(quads):
                        w = len(quad)
                        s_ps = sps.tile([HD, 4, PAIR], F32, tag="s")
                        for t, (c, mi) in enumerate(quad):
                            nc.tensor.matmul(
                                s_ps[:, t, :],
                                qk_sb[:, HPC + h, c * HD:(c + 1) * HD],
                                qk_sb[:, h, qsl],
                                start=True,
                                stop=True,
                            )
                        nc.scalar.activation(
                            pt[:, qi * 4:qi * 4 + w, :], s_ps[:, 0:w, :], EXP
                        )
                        # multiplicative exp-mask applied to pt
                        # (exp(s+m) == exp(s)*exp(m)), off the exp chain
                        t = 0
                        while t < w:
                            c, mi = quad[t]
                            if mi < 0:
                                t += 1
                                continue
                            r = t + 1
                            while (preload_mask and r < w and quad[r][1] >= 0
                                   and quad[r][1] == quad[r - 1][1] + 1):
                                r += 1
                            if preload_mask:
                                sl = slice(qi * 4 + t, qi * 4 + r)
                                nc.vector.tensor_mul(
                                    pt[:, sl, :], pt[:, sl, :],
                                    mk_sb[:, mi:mi + (r - t), :],
                                )
                            else:
                                sl = slice(qi * 4 + t, qi * 4 + t + 1)
                                nc.vector.tensor_mul(
                                    pt[:, sl, :], pt[:, sl, :], mask_tile(mi)
                                )
                                r = t + 1
                            t = r
                        if pending_ol is not None:
                            if emit_ol(pending_ol):
                                pending_fin = pending_ol[2]
                            pending_ol = None
                        if pending_fin is not None and pending_fin is not st:
                            finalize(pending_fin)
                            pending_fin = None
                        pending_ol = (qi * 4, quad, st)
            if pending_ol is not None:
                if emit_ol(pending_ol):
                    pending_fin = pending_ol[2]
            if pending_fin is not None:
                finalize(pending_fin)
        # ---- output projection ----
        with ExitStack() as ph:
            evp = ph.enter_context(tc.tile_pool(name="evp", side="right", bufs=4))
            wops = ph.enter_context(tc.tile_pool(name="wops", bufs=4, space="PSUM"))
            for tck in range(NCHUNK):
                tsl = slice(tck * HD, (tck + 1) * HD)
                for es in range(4):
                    esl = slice(es * 512, (es + 1) * 512)
                    ps = wops.tile([HD, 512], F32, tag="wo")
                    for h in range(HPC):
                        nc.tensor.matmul(
                            ps[:],
                            ctx_sb[:, h, tsl],
                            wo_sb[:, h, esl],
                            start=(h == 0),
                            stop=(h == HPC - 1),
                        )
                    ev = evp.tile([HD, 512], F32, tag="ev")
                    nc.scalar.copy(ev[:], ps[:])
                    nc.sync.dma_start(out_d[tsl, esl], ev[:])
    nc.compile()
    return nc


_PERM = np.concatenate(
    [np.concatenate([np.arange(0, HD, 2), np.arange(1, HD, 2)]) + h * HD
     for h in range(HPC)]
)


def prepare(x, freqs, mask, wq, wk, wv, wo):
    """Host-side sharding/prep. Returns (nc, in_maps)."""
    x = np.asarray(x, np.float32)
    freqs = np.asarray(freqs, np.float32)
    mask = np.asarray(mask, np.float32)
    wq, wk, wv, wo = (np.asarray(w, np.float32) for w in (wq, wk, wv, wo))

    statuses, maskt = _mask_structure(mask)
    nc = _build_program(statuses, maskt.shape[1])

    scale = np.float32(1.0 / np.sqrt(HD))
    cos = np.ascontiguousarray(freqs[:, :, 0].T)  # (64, T)
    sin = np.ascontiguousarray(freqs[:, :, 1].T)
    cs = np.empty((HD, 2, T), np.float32)
    cs[0:64, 0, :] = cos
    cs[64:128, 0, :] = cos
    cs[0:64, 1, :] = -sin
    cs[64:128, 1, :] = sin

    ones_col = np.ones((HD, 1), np.float32)
    ones_row = np.ones((1, HD), np.float32)
    xt = [np.ascontiguousarray(x[b].T) for b in range(B)]

    in_maps = []
    for core in range(8):
        b, g = core // 4, core % 4
        cols = slice(g * HPC * HD, (g + 1) * HPC * HD)
        in_maps.append({
            "xt": xt[b],
            "wqt": np.ascontiguousarray((wq.T[:, cols] * scale)[:, _PERM]),
            "wkt": np.ascontiguousarray(wk.T[:, cols][:, _PERM]),
            "wvt": np.ascontiguousarray(wv.T[:, cols]),
            "wot": np.ascontiguousarray(wo.T[cols, :]),
            "cs": cs,
            "maskt": maskt,
            "ones_col": ones_col,
            "ones_row": ones_row,
        })
    return nc, in_maps


def run(x, freqs, mask, wq, wk, wv, wo, **spmd_kwargs):
    nc, in_maps = prepare(x, freqs, mask, wq, wk, wv, wo)
    res = run_bass_kernel_spmd(nc, in_maps, list(range(8)), **spmd_kwargs)
    parts = [res.results[c]["out"] for c in range(8)]
    out = np.stack([
        parts[b * 4] + parts[b * 4 + 1] + parts[b * 4 + 2] + parts[b * 4 + 3]
        for b in range(B)
    ]).astype(np.float32)
    return out, res


def kernel(x, freqs, mask, wq, wk, wv, wo):
    out, _ = run(x, freqs, mask, wq, wk, wv, wo)
    return out
